# revision 16
# baseline (speedup 1.0000x reference)
"""Locality (2D-window) self-attention kernel on a single Trainium2 NeuronCore.

Problem: B=2, N=4096 (64x64 grid), DIM=256, 8 heads x 32, window 7x7.
  qkv = x @ W_qkv.T ; per-head local attention with 2D grid mask;
  out = attn_out @ W_proj.T + b_proj.

Why one core: each kernel dispatch through the axon PJRT client costs ~1-2 ms
of RPC latency *per participating core* (measured: 8-core ~9-12 ms/call,
1-core ~2 ms/call), while the whole problem is only ~0.3 ms of device time.
Packing everything onto core 0 minimizes wall-clock per call. Per-call cost
also grows ~50 us per argument and ~12 us/MB of plain input (~5 us/MB when
the input is a donated in-out tensor with a declared custom-call alias), so
all inputs are packed host-side into ONE bf16 blob passed as an aliased
in-out tensor; the only other argument is the donated f32 output.

Device program (per batch b = 0, 1; buffers reused across batches):
  phase 1: qT [hd, 4096], kT [hd, 4480] (transposed, 3-grid-row zero halo on
           both ends) and v_aug [128, 33] per 128-token chunk per head
           (col 32 = 1.0 -> attention row-sums fall out of the AV matmul).
  phase 2: per 128-query tile: per head-pair scores^T chunks via PE (K=32)
           into PSUM, exp on ACT into slices of one [128, 4096] tile; ONE
           window-mask multiply per tile on DVE (in2 repeats the [128,1024]
           mask across head pairs with a stride-0 AP; masks SBUF-resident:
           only 5 distinct patterns across all 32 tiles); P^T @ v_aug on PE
           (contraction over keys on partitions - no P transpose needed),
           per-partition normalize on DVE, then per tile: PE transpose of
           the [128, 256] head-concat output and the final W_proj matmul.
           PSUM->SBUF copies run on the otherwise-idle GPSIMD engine.

Scale (hd^-0.5 * temperature) is folded into the Q weights on the host.
Softmax skips the max-subtraction (scores are O(1) by construction:
exp stays in fp32 range), matching jax softmax to ~1e-6. Zero-padded halo
tokens produce k=0 -> score 0 -> exp 1, removed by the mask.
"""

import numpy as np

import concourse.bass as bass
import concourse.bacc as bacc
import concourse.tile as tile
from concourse import mybir

F32 = mybir.dt.float32
BF16 = mybir.dt.bfloat16

B, N, DIM = 2, 4096, 256
H, HD = 8, 32
GRID = 64
HALF = 3  # window 7 // 2
SCALE = HD ** -0.5

NT = N // 128          # 32 query tiles per batch
PAD = HALF * GRID      # 192 zero tokens of halo on each end
NHB = N + 2 * PAD      # 4480 padded tokens per batch
NCH = NHB // 128       # 35 key/value chunks per batch
MREP = (0, 1, 2, 30, 31)  # representative tiles for the 5 mask classes

# blob column layout (all bf16, [128, BLOB_COLS])
OFF_WQKV = 0                     # [256,768] as cc-blocks of [128,768]
OFF_WP = OFF_WQKV + 2 * 768      # [256,256] as cc-blocks of [128,256]
OFF_BP = OFF_WP + 2 * 256        # [1,256] in partition row 0
OFF_ID = OFF_BP + 256            # [128,128] identity
OFF_MSK = OFF_ID + 128           # [128, 5*1024] masks
OFF_X = OFF_MSK + 5 * 1024       # x: 2 cc-blocks of [128, XCOLS]
XSTEP = N + PAD                  # per-batch stride inside a cc block
XCOLS = B * XSTEP + PAD          # 8768: pad | x0 | pad | x1 | pad
BLOB_COLS = OFF_X + 2 * XCOLS    # 25088


def _mask_class(t: int) -> int:
    return {0: 0, 1: 1, 30: 3, 31: 4}.get(t, 2)


def _build_program() -> bass.Bass:
    nc = bacc.Bacc("TRN2")

    blob = nc.declare_dram_parameter("blob", [128, BLOB_COLS], BF16, isOutput=True)
    y = nc.declare_dram_parameter("y", [B * N, DIM], F32, isOutput=True)

    with tile.TileContext(nc) as tc:
        with (
            tc.tile_pool(name="persist", bufs=1) as pp,
            tc.tile_pool(name="work", bufs=2) as wk,
            tc.tile_pool(name="outs", bufs=2) as op,
            tc.tile_pool(name="ps_s", bufs=2, space="PSUM") as ps_s,
            tc.tile_pool(name="ph1", bufs=1, space="PSUM") as ph1,
            tc.tile_pool(name="ps_av", bufs=2, space="PSUM") as ps_av,
            tc.tile_pool(name="ps_y", bufs=1, space="PSUM") as ps_y,
        ):
            # ---- constants (one DMA each from the blob) ----
            wq = []
            for cc in range(2):
                t = pp.tile([128, 3 * DIM], BF16, name=f"wq{cc}", tag=f"wq{cc}")
                nc.sync.dma_start(
                    out=t, in_=blob[:, OFF_WQKV + cc * 768:OFF_WQKV + (cc + 1) * 768])
                wq.append(t)
            wp = []
            for cc in range(2):
                t = pp.tile([128, DIM], BF16, name=f"wp{cc}", tag=f"wp{cc}")
                nc.sync.dma_start(
                    out=t, in_=blob[:, OFF_WP + cc * 256:OFF_WP + (cc + 1) * 256])
                wp.append(t)
            bb = pp.tile([1, DIM], BF16, name="bb", tag="bb")
            nc.sync.dma_start(out=bb, in_=blob[0:1, OFF_BP:OFF_BP + 256])
            idt = pp.tile([128, 128], BF16, name="idt", tag="idt")
            nc.sync.dma_start(out=idt, in_=blob[:, OFF_ID:OFF_ID + 128])
            ones = pp.tile([1, 128], BF16, name="ones", tag="ones")
            nc.gpsimd.memset(ones, 1.0)
            msk = pp.tile([128, 5 * 1024], BF16, name="msk", tag="msk")
            nc.sync.dma_start(out=msk, in_=blob[:, OFF_MSK:OFF_MSK + 5 * 1024])

            for b in range(B):
                # ---- load x for this batch (chunked so phase 1 can stream) ----
                xs = []
                for cc in range(2):
                    t = pp.tile([128, NHB], BF16, name=f"xs{cc}", tag=f"xs{cc}")
                    xs.append(t)
                for n0 in range(0, NHB, 512):
                    nn = min(512, NHB - n0)
                    for cc in range(2):
                        src0 = OFF_X + cc * XCOLS + b * XSTEP + n0
                        nc.sync.dma_start(
                            out=xs[cc][:, n0:n0 + nn],
                            in_=blob[:, src0:src0 + nn])

                # ---- software pipeline: phase-1 in 512-token groups,
                # phase-2 tiles emitted as soon as their groups are ready.
                # phase-1 matmuls use their own PSUM pool so they never flush
                # the score-tile rotation that feeds ACT's exp pipeline. ----
                qt = {}   # (pg, g) -> [128, 512] queries group
                kt = {}   # (pg, g) -> [128, 512|384] keys group
                qX = {}   # g -> [64, 512] (offset-96 heads, pg-stacked)
                kX = {}
                vv = {}   # ch -> [128, H*(HD+1)] v_aug chunk

                def emit_k_group(g):
                    w = min(512, NHB - 512 * g)
                    for pg in range(2):
                        tl = pp.tile([128, w], BF16, name=f"kT{pg}_{g}",
                                     tag=f"kT{pg}_{g}")
                        ps = ph1.tile([128, 512], F32, name="ps1k", tag="ps1")
                        for cc in range(2):
                            nc.tensor.matmul(
                                out=ps[:, :w],
                                lhsT=wq[cc][:, DIM + pg * 128:DIM + pg * 128 + 128],
                                rhs=xs[cc][:, 512 * g:512 * g + w],
                                start=(cc == 0), stop=(cc == 1),
                            )
                        nc.vector.tensor_copy(out=tl, in_=ps[:, :w])
                        kt[(pg, g)] = tl
                    tx = pp.tile([64, w], BF16, name=f"kX_{g}", tag=f"kX_{g}")
                    for pg in range(2):
                        nc.vector.tensor_copy(
                            out=tx[pg * 32:(pg + 1) * 32, :],
                            in_=kt[(pg, g)][96:128, :])
                    kX[g] = tx

                def emit_q_group(g):
                    for pg in range(2):
                        tl = pp.tile([128, 512], BF16, name=f"qT{pg}_{g}",
                                     tag=f"qT{pg}_{g}")
                        ps = ph1.tile([128, 512], F32, name="ps1q", tag="ps1")
                        for cc in range(2):
                            nc.tensor.matmul(
                                out=ps,
                                lhsT=wq[cc][:, pg * 128:pg * 128 + 128],
                                rhs=xs[cc][:, PAD + 512 * g:PAD + 512 * g + 512],
                                start=(cc == 0), stop=(cc == 1),
                            )
                        nc.scalar.copy(out=tl, in_=ps)
                        qt[(pg, g)] = tl
                    tx = pp.tile([64, 512], BF16, name=f"qX_{g}", tag=f"qX_{g}")
                    for pg in range(2):
                        nc.vector.tensor_copy(
                            out=tx[pg * 32:(pg + 1) * 32, :],
                            in_=qt[(pg, g)][96:128, :])
                    qX[g] = tx

                def emit_v_chunk(ch):
                    vt = pp.tile([128, H * (HD + 1)], BF16,
                                 name=f"vv{ch}", tag=f"vv{ch}")
                    ps = ph1.tile([128, DIM], F32, name="ps1v", tag="ps1")
                    for cc in range(2):
                        nc.tensor.matmul(
                            out=ps,
                            lhsT=xs[cc][:, ch * 128:ch * 128 + 128],
                            rhs=wq[cc][:, 2 * DIM:3 * DIM],
                            start=(cc == 0), stop=(cc == 1),
                        )
                    v3 = vt.rearrange("p (h e) -> p h e", e=HD + 1)
                    nc.scalar.copy(
                        out=v3[:, :, 0:HD],
                        in_=ps.rearrange("p (h d) -> p h d", d=HD),
                    )
                    nc.gpsimd.memset(v3[:, :, HD:HD + 1], 1.0)
                    vv[ch] = vt

                def emit_tile(t):
                    cls = _mask_class(t)
                    oall = op.tile([128, DIM], BF16, name="oall", tag="oall")
                    pe_t = wk.tile([128, 4 * 1024], BF16, name="pe_t", tag="pe_t")
                    pT = wk.tile([128, 4 * 1024], BF16, name="pT", tag="pT")
                    for hp in range(H // 2):
                        # scores for a PAIR of heads into one 2-bank PSUM tile
                        sps = ps_s.tile([128, 1024], F32, name="sps", tag="sps")
                        for hi in range(2):
                            h = 2 * hp + hi
                            pg, r = h // 4, (h % 4) * HD
                            for j in range(4):
                                c = t + j
                                if r == 96:
                                    ksrc = kX[c // 4][pg * 32:pg * 32 + HD,
                                                      128 * (c % 4):
                                                      128 * (c % 4) + 128]
                                    qsrc = qX[t // 4][pg * 32:pg * 32 + HD,
                                                      128 * (t % 4):
                                                      128 * (t % 4) + 128]
                                else:
                                    ksrc = kt[(pg, c // 4)][r:r + HD,
                                                            128 * (c % 4):
                                                            128 * (c % 4) + 128]
                                    qsrc = qt[(pg, t // 4)][r:r + HD,
                                                            128 * (t % 4):
                                                            128 * (t % 4) + 128]
                                nc.tensor.matmul(
                                    out=sps[:, hi * 512 + j * 128:
                                            hi * 512 + (j + 1) * 128],
                                    lhsT=ksrc, rhs=qsrc,
                                    start=True, stop=True,
                                )
                        nc.scalar.activation(
                            out=pe_t[:, hp * 1024:(hp + 1) * 1024], in_=sps,
                            func=mybir.ActivationFunctionType.Exp,
                        )
                    # ONE masked multiply for all 4 head pairs: in2 repeats the
                    # class mask across pairs via a stride-0 middle AP dim
                    mk_ap = msk[:, cls * 1024:(cls + 1) * 1024]
                    nc.vector.tensor_mul(
                        pT.rearrange("p (g c) -> p g c", c=1024),
                        pe_t.rearrange("p (g c) -> p g c", c=1024),
                        bass.AP(tensor=mk_ap.tensor, offset=mk_ap.offset,
                                ap=[list(mk_ap.ap[0]), [0, 4], [1, 1024]]))
                    for hp2 in range(2):
                        # FOUR heads' AV into one PSUM bank: hh*(HD+1) col base
                        av = ps_av.tile([128, 4 * (HD + 1)], F32, name="av",
                                        tag="av")
                        for hh in range(4):
                            h = 4 * hp2 + hh
                            hp, hi = h // 2, h % 2
                            for j in range(4):
                                nc.tensor.matmul(
                                    out=av[:, hh * (HD + 1):hh * (HD + 1) + HD + 1],
                                    lhsT=pT[:, hp * 1024 + hi * 512 + j * 128:
                                            hp * 1024 + hi * 512 + (j + 1) * 128],
                                    rhs=vv[t + j][:, h * (HD + 1):
                                                  (h + 1) * (HD + 1)],
                                    start=(j == 0), stop=(j == 3),
                                )
                        # one recip over 4 rowsums, one broadcast-mul normalize
                        rec = wk.tile([128, 4], F32, name="rec", tag="rec")
                        nc.vector.reciprocal(
                            rec,
                            bass.AP(tensor=av.tensor, offset=av.offset + HD,
                                    ap=[list(av.ap[0]), [HD + 1, 4]]))
                        nc.vector.tensor_mul(
                            oall[:, hp2 * 128:(hp2 + 1) * 128]
                                .rearrange("p (g d) -> p g d", d=HD),
                            av.rearrange("p (g e) -> p g e", e=HD + 1)[:, :, 0:HD],
                            bass.AP(tensor=rec.tensor, offset=rec.offset,
                                    ap=[list(rec.ap[0]), [1, 4], [0, HD]]))
                    ypst = ps_y.tile([128, DIM + 128], F32, name="yps", tag="psy")
                    yps = ypst[:, 0:DIM]
                    tp = ypst[:, DIM:DIM + 128].bitcast(BF16)
                    for cg in range(2):
                        nc.tensor.transpose(
                            tp[:, cg * 128:(cg + 1) * 128],
                            oall[:, cg * 128:(cg + 1) * 128], idt)
                    oT = op.tile([128, 256], BF16, name="oT", tag="oT")
                    nc.vector.tensor_copy(out=oT, in_=tp)
                    nc.tensor.matmul(out=yps, lhsT=ones,
                                     rhs=bb[0:1, :],
                                     start=True, stop=False)
                    for cg in range(2):
                        nc.tensor.matmul(
                            out=yps,
                            lhsT=oT[:, cg * 128:(cg + 1) * 128],
                            rhs=wp[cg],
                            start=False, stop=(cg == 1),
                        )
                    yt = op.tile([128, DIM], F32, name="yt", tag="yt")
                    nc.vector.tensor_copy(out=yt, in_=yps)
                    nc.sync.dma_start(
                        out=y[b * N + t * 128:b * N + (t + 1) * 128, :], in_=yt)

                for g in range((NHB + 511) // 512):  # 9 groups
                    emit_k_group(g)
                    if g < N // 512:
                        emit_q_group(g)
                    for ch in range(4 * g, min(4 * g + 4, NCH)):
                        emit_v_chunk(ch)
                    if g >= 1:
                        for t in range(4 * (g - 1), 4 * g):
                            emit_tile(t)

    nc.compile()  # legalize waits (<=1 per instruction) for walrus
    return nc


_PROGRAM_CACHE: dict = {}


def _program() -> bass.Bass:
    if "nc" not in _PROGRAM_CACHE:
        _PROGRAM_CACHE["nc"] = _build_program()
    return _PROGRAM_CACHE["nc"]


def _masks() -> np.ndarray:
    """maskP[ki, cls*1024 + j*128 + qi] (duplicated at +512 for head pairs)."""
    import ml_dtypes
    m = np.zeros((128, 5 * 1024), ml_dtypes.bfloat16)
    ki = np.arange(128)
    qi = np.arange(128)
    kc = ki % GRID
    qc = qi % GRID
    for cls, t in enumerate(MREP):
        qr = 2 * t + qi // GRID
        for j in range(4):
            kr = 2 * (t + j) + ki // GRID - HALF
            valid = (
                (kr[:, None] >= 0) & (kr[:, None] < GRID)
                & (np.abs(kr[:, None] - qr[None, :]) <= HALF)
                & (np.abs(kc[:, None] - qc[None, :]) <= HALF)
            )
            m[:, cls * 1024 + j * 128:cls * 1024 + (j + 1) * 128] = valid
            m[:, cls * 1024 + 512 + j * 128:
              cls * 1024 + 512 + (j + 1) * 128] = valid
    return m


def _in_blob(x, W_qkv, W_proj, b_proj, temperature) -> np.ndarray:
    import ml_dtypes
    bf = ml_dtypes.bfloat16
    x = np.asarray(x, np.float32)
    wqkvT = np.ascontiguousarray(np.asarray(W_qkv, np.float32).T)
    wqkvT[:, :DIM] *= np.float32(SCALE) * np.float32(np.asarray(temperature)[0])

    blob = np.zeros((128, BLOB_COLS), bf)
    for cc in range(2):
        blob[:, OFF_WQKV + cc * 768:OFF_WQKV + (cc + 1) * 768] = \
            wqkvT[cc * 128:(cc + 1) * 128].astype(bf)
        blob[:, OFF_WP + cc * 256:OFF_WP + (cc + 1) * 256] = \
            np.asarray(W_proj, np.float32).T[cc * 128:(cc + 1) * 128].astype(bf)
    blob[0, OFF_BP:OFF_BP + 256] = np.asarray(b_proj, np.float32).astype(bf)
    blob[:, OFF_ID:OFF_ID + 128] = np.eye(128, dtype=np.float32)
    blob[:, OFF_MSK:OFF_MSK + 5 * 1024] = _masks()
    for b in range(B):
        xT = x[b].T.astype(bf)  # [256, 4096]
        for cc in range(2):
            c0 = OFF_X + cc * XCOLS + b * XSTEP + PAD
            blob[:, c0:c0 + N] = xT[cc * 128:(cc + 1) * 128]
    return blob


class _Runner:
    """Persistent single-core PJRT executable (1-device shard_map, mirroring
    bass2jax.run_bass_via_pjrt's multi-core path so the jit cache survives
    across calls; the plain-jit single-core path wedges the exec unit).
    All tensors are donated: blob is an aliased in-out (content readable on
    device, no per-call XLA copy), y is a plain donated output."""

    NCORES = 1

    def __init__(self, nc: bass.Bass):
        import jax
        from jax.experimental.shard_map import shard_map
        from jax.sharding import Mesh, PartitionSpec
        from concourse import bass2jax
        from concourse import mybir as mb

        bass2jax.install_neuronx_cc_hook()
        self.jax = jax

        partition_name = (nc.partition_id_tensor.name
                          if nc.partition_id_tensor else None)
        in_names, out_names, out_avals, zero_outs = [], [], [], []
        for alloc in nc.m.functions[0].allocations:
            if not isinstance(alloc, mb.MemoryLocationSet):
                continue
            name = alloc.memorylocations[0].name
            if alloc.kind == "ExternalInput":
                if name != partition_name:
                    in_names.append(name)
            elif alloc.kind == "ExternalOutput":
                out_names.append(name)
                shape = tuple(alloc.tensor_shape)
                dtype = mb.dt.np(alloc.dtype)
                out_avals.append(jax.core.ShapedArray(shape, dtype))
                zero_outs.append(np.zeros(shape, dtype))
        self.in_names, self.out_names = in_names, out_names
        self.out_avals, self.zero_outs = out_avals, zero_outs
        n_params, n_outs = len(in_names), len(out_names)
        all_names = list(in_names + out_names)
        if partition_name is not None:
            all_names.append(partition_name)
        all_names = tuple(all_names)
        # every output aliases the operand that carries its donated buffer
        aliases = tuple((oi, n_params + oi) for oi in range(n_outs))

        def _body(*args):
            operands = list(args)
            if partition_name is not None:
                operands.append(bass2jax.partition_id_tensor())
            outs = bass2jax._bass_exec_p.bind(
                *operands,
                out_avals=tuple(out_avals),
                in_names=all_names,
                out_names=tuple(out_names),
                lowering_input_output_aliases=aliases,
                sim_require_finite=True,
                sim_require_nnan=True,
                nc=nc,
            )
            return tuple(outs)

        devices = jax.devices()[:self.NCORES]
        self.mesh = Mesh(np.asarray(devices), ("core",))
        in_specs = (PartitionSpec("core"),) * (n_params + n_outs)
        out_specs = (PartitionSpec("core"),) * n_outs
        self.sharded = jax.jit(
            shard_map(_body, mesh=self.mesh, in_specs=in_specs,
                      out_specs=out_specs, check_rep=False),
            donate_argnums=tuple(range(n_params, n_params + n_outs)),
            keep_unused=True,
        )

    def _operands(self, blob):
        """Donated operands in out_names order (blob data + zeroed outputs)."""
        ops = []
        for n, z in zip(self.out_names, self.zero_outs):
            ops.append(np.ascontiguousarray(blob) if n == "blob"
                       else np.zeros(z.shape, z.dtype))
        return ops

    def __call__(self, blob):
        out_arrs = self.sharded(*self._operands(blob))
        return {n: np.asarray(out_arrs[i]) for i, n in enumerate(self.out_names)}

    def bench(self, blob, iters: int = 200):
        """Steady-state per-iteration time (s) with pipelined dispatch."""
        import time
        jax = self.jax
        zs = [[jax.device_put(o) for o in self._operands(blob)]
              for _ in range(iters + 2)]
        for zz in zs:
            for z in zz:
                z.block_until_ready()
        # warmup
        outs = self.sharded(*zs[0])
        jax.block_until_ready(outs)
        outs = self.sharded(*zs[1])
        jax.block_until_ready(outs)
        t0 = time.monotonic()
        last = None
        for i in range(iters):
            last = self.sharded(*zs[2 + i])
        jax.block_until_ready(last)
        t1 = time.monotonic()
        return (t1 - t0) / iters


def _runner() -> _Runner:
    if "runner" not in _PROGRAM_CACHE:
        _PROGRAM_CACHE["runner"] = _Runner(_program())
    return _PROGRAM_CACHE["runner"]


def run(inputs: dict):
    """Returns (out [B,N,DIM] f32, raw result dict)."""
    blob = _in_blob(**inputs)
    result = _runner()(blob)
    out = np.ascontiguousarray(result["y"].reshape(B, N, DIM))
    return out, result


def kernel(x, W_qkv, W_proj, b_proj, temperature):
    out, _ = run({"x": x, "W_qkv": W_qkv, "W_proj": W_proj,
                  "b_proj": b_proj, "temperature": temperature})
    return out


# revision 17
# speedup vs baseline: 1.0438x; 1.0438x over previous
"""Locality (2D-window) self-attention kernel on a single Trainium2 NeuronCore.

Problem: B=2, N=4096 (64x64 grid), DIM=256, 8 heads x 32, window 7x7.
  qkv = x @ W_qkv.T ; per-head local attention with 2D grid mask;
  out = attn_out @ W_proj.T + b_proj.

Why one core: each kernel dispatch through the axon PJRT client costs ~1-2 ms
of RPC latency *per participating core* (measured: 8-core ~9-12 ms/call,
1-core ~2 ms/call), while the whole problem is only ~0.3 ms of device time.
Packing everything onto core 0 minimizes wall-clock per call. Per-call cost
also grows ~50 us per argument and ~12 us/MB of plain input (~5 us/MB when
the input is a donated in-out tensor with a declared custom-call alias), so
all inputs are packed host-side into ONE bf16 blob passed as an aliased
in-out tensor; the only other argument is the donated f32 output.

Device program (per batch b = 0, 1; buffers reused across batches):
  phase 1: qT [hd, 4096], kT [hd, 4480] (transposed, 3-grid-row zero halo on
           both ends) and v_aug [128, 33] per 128-token chunk per head
           (col 32 = 1.0 -> attention row-sums fall out of the AV matmul).
  phase 2: per 128-query tile: per head-pair scores^T chunks via PE (K=32)
           into PSUM, exp on ACT into slices of one [128, 4096] tile; ONE
           window-mask multiply per tile on DVE (in2 repeats the [128,1024]
           mask across head pairs with a stride-0 AP; masks SBUF-resident:
           only 5 distinct patterns across all 32 tiles); P^T @ v_aug on PE
           (contraction over keys on partitions - no P transpose needed),
           per-partition normalize on DVE, then per tile: PE transpose of
           the [128, 256] head-concat output and the final W_proj matmul.
           PSUM->SBUF copies run on the otherwise-idle GPSIMD engine.

Scale (hd^-0.5 * temperature) is folded into the Q weights on the host.
Softmax skips the max-subtraction (scores are O(1) by construction:
exp stays in fp32 range), matching jax softmax to ~1e-6. Zero-padded halo
tokens produce k=0 -> score 0 -> exp 1, removed by the mask.
"""

import numpy as np

import concourse.bass as bass
import concourse.bacc as bacc
import concourse.tile as tile
from concourse import mybir

F32 = mybir.dt.float32
BF16 = mybir.dt.bfloat16

B, N, DIM = 2, 4096, 256
H, HD = 8, 32
GRID = 64
HALF = 3  # window 7 // 2
SCALE = HD ** -0.5

NT = N // 128          # 32 query tiles per batch
PAD = HALF * GRID      # 192 zero tokens of halo on each end
NHB = N + 2 * PAD      # 4480 padded tokens per batch
NCH = NHB // 128       # 35 key/value chunks per batch
MREP = (0, 1, 2, 30, 31)  # representative tiles for the 5 mask classes

# blob column layout (all bf16, [128, BLOB_COLS])
OFF_WQKV = 0                     # [256,768] as cc-blocks of [128,768]
OFF_WP = OFF_WQKV + 2 * 768      # [256,256] as cc-blocks of [128,256]
OFF_BP = OFF_WP + 2 * 256        # [1,256] in partition row 0
OFF_ID = OFF_BP + 256            # [128,128] identity
OFF_MSK = OFF_ID + 128           # [128, 5*1024] masks
OFF_X = OFF_MSK + 5 * 1024       # x: 2 cc-blocks of [128, XCOLS]
XSTEP = N + PAD                  # per-batch stride inside a cc block
XCOLS = B * XSTEP + PAD          # 8768: pad | x0 | pad | x1 | pad
BLOB_COLS = OFF_X + 2 * XCOLS    # 25088


def _mask_class(t: int) -> int:
    return {0: 0, 1: 1, 30: 3, 31: 4}.get(t, 2)


def _build_program() -> bass.Bass:
    nc = bacc.Bacc("TRN2")

    blob = nc.declare_dram_parameter("blob", [128, BLOB_COLS], BF16, isOutput=True)
    y = nc.declare_dram_parameter("y", [B * N, DIM], F32, isOutput=True)

    with tile.TileContext(nc) as tc:
        with (
            tc.tile_pool(name="persist", bufs=1) as pp,
            tc.tile_pool(name="work", bufs=2) as wk,
            tc.tile_pool(name="outs", bufs=2) as op,
            tc.tile_pool(name="ps_s", bufs=2, space="PSUM") as ps_s,
            tc.tile_pool(name="ph1", bufs=1, space="PSUM") as ph1,
            tc.tile_pool(name="ps_av", bufs=2, space="PSUM") as ps_av,
            tc.tile_pool(name="ps_y", bufs=1, space="PSUM") as ps_y,
        ):
            # ---- constants (one DMA each from the blob) ----
            wq = []
            for cc in range(2):
                t = pp.tile([128, 3 * DIM], BF16, name=f"wq{cc}", tag=f"wq{cc}")
                nc.sync.dma_start(
                    out=t, in_=blob[:, OFF_WQKV + cc * 768:OFF_WQKV + (cc + 1) * 768])
                wq.append(t)
            wp = []
            for cc in range(2):
                t = pp.tile([128, DIM], BF16, name=f"wp{cc}", tag=f"wp{cc}")
                nc.sync.dma_start(
                    out=t, in_=blob[:, OFF_WP + cc * 256:OFF_WP + (cc + 1) * 256])
                wp.append(t)
            bb = pp.tile([1, DIM], BF16, name="bb", tag="bb")
            nc.sync.dma_start(out=bb, in_=blob[0:1, OFF_BP:OFF_BP + 256])
            idt = pp.tile([128, 128], BF16, name="idt", tag="idt")
            nc.sync.dma_start(out=idt, in_=blob[:, OFF_ID:OFF_ID + 128])
            ones = pp.tile([1, 128], BF16, name="ones", tag="ones")
            nc.gpsimd.memset(ones, 1.0)
            msk = pp.tile([128, 5 * 1024], BF16, name="msk", tag="msk")
            nc.sync.dma_start(out=msk, in_=blob[:, OFF_MSK:OFF_MSK + 5 * 1024])

            for b in range(B):
                # ---- load x for this batch (chunked so phase 1 can stream) ----
                xs = []
                for cc in range(2):
                    t = pp.tile([128, NHB], BF16, name=f"xs{cc}", tag=f"xs{cc}")
                    xs.append(t)
                for n0 in range(0, NHB, 512):
                    nn = min(512, NHB - n0)
                    for cc in range(2):
                        src0 = OFF_X + cc * XCOLS + b * XSTEP + n0
                        nc.sync.dma_start(
                            out=xs[cc][:, n0:n0 + nn],
                            in_=blob[:, src0:src0 + nn])

                # ---- software pipeline: phase-1 in 512-token groups,
                # phase-2 tiles emitted as soon as their groups are ready.
                # phase-1 matmuls use their own PSUM pool so they never flush
                # the score-tile rotation that feeds ACT's exp pipeline. ----
                qt = {}   # (pg, g) -> [128, 512] queries group
                kt = {}   # (pg, g) -> [128, 512|384] keys group
                qX = {}   # g -> [64, 512] (offset-96 heads, pg-stacked)
                kX = {}
                vv = {}   # ch -> [128, H*(HD+1)] v_aug chunk

                def emit_k_group(g):
                    w = min(512, NHB - 512 * g)
                    for pg in range(2):
                        tl = pp.tile([128, w], BF16, name=f"kT{pg}_{g}",
                                     tag=f"kT{pg}_{g}")
                        ps = ph1.tile([128, 512], F32, name="ps1k", tag="ps1")
                        for cc in range(2):
                            nc.tensor.matmul(
                                out=ps[:, :w],
                                lhsT=wq[cc][:, DIM + pg * 128:DIM + pg * 128 + 128],
                                rhs=xs[cc][:, 512 * g:512 * g + w],
                                start=(cc == 0), stop=(cc == 1),
                            )
                        nc.vector.tensor_copy(out=tl, in_=ps[:, :w])
                        kt[(pg, g)] = tl
                    tx = pp.tile([64, w], BF16, name=f"kX_{g}", tag=f"kX_{g}")
                    for pg in range(2):
                        nc.vector.tensor_copy(
                            out=tx[pg * 32:(pg + 1) * 32, :],
                            in_=kt[(pg, g)][96:128, :])
                    kX[g] = tx

                def emit_q_group(g):
                    for pg in range(2):
                        tl = pp.tile([128, 512], BF16, name=f"qT{pg}_{g}",
                                     tag=f"qT{pg}_{g}")
                        ps = ph1.tile([128, 512], F32, name="ps1q", tag="ps1")
                        for cc in range(2):
                            nc.tensor.matmul(
                                out=ps,
                                lhsT=wq[cc][:, pg * 128:pg * 128 + 128],
                                rhs=xs[cc][:, PAD + 512 * g:PAD + 512 * g + 512],
                                start=(cc == 0), stop=(cc == 1),
                            )
                        nc.scalar.copy(out=tl, in_=ps)
                        qt[(pg, g)] = tl
                    tx = pp.tile([64, 512], BF16, name=f"qX_{g}", tag=f"qX_{g}")
                    for pg in range(2):
                        nc.vector.tensor_copy(
                            out=tx[pg * 32:(pg + 1) * 32, :],
                            in_=qt[(pg, g)][96:128, :])
                    qX[g] = tx

                def emit_v_chunk(ch):
                    vt = pp.tile([128, H * (HD + 1)], BF16,
                                 name=f"vv{ch}", tag=f"vv{ch}")
                    ps = ph1.tile([128, DIM], F32, name="ps1v", tag="ps1")
                    for cc in range(2):
                        nc.tensor.matmul(
                            out=ps,
                            lhsT=xs[cc][:, ch * 128:ch * 128 + 128],
                            rhs=wq[cc][:, 2 * DIM:3 * DIM],
                            start=(cc == 0), stop=(cc == 1),
                        )
                    v3 = vt.rearrange("p (h e) -> p h e", e=HD + 1)
                    nc.scalar.copy(
                        out=v3[:, :, 0:HD],
                        in_=ps.rearrange("p (h d) -> p h d", d=HD),
                    )
                    nc.gpsimd.memset(v3[:, :, HD:HD + 1], 1.0)
                    vv[ch] = vt

                def emit_tile(t):
                    cls = _mask_class(t)
                    oall = op.tile([128, DIM], BF16, name="oall", tag="oall")
                    pe_t = wk.tile([128, 4 * 1024], BF16, name="pe_t", tag="pe_t")
                    pT = wk.tile([128, 4 * 1024], BF16, name="pT", tag="pT")
                    for hp in range(H // 2):
                        # scores for a PAIR of heads into one 2-bank PSUM tile
                        sps = ps_s.tile([128, 1024], F32, name="sps", tag="sps")
                        for hi in range(2):
                            h = 2 * hp + hi
                            pg, r = h // 4, (h % 4) * HD
                            for j in range(4):
                                c = t + j
                                if r == 96:
                                    ksrc = kX[c // 4][pg * 32:pg * 32 + HD,
                                                      128 * (c % 4):
                                                      128 * (c % 4) + 128]
                                    qsrc = qX[t // 4][pg * 32:pg * 32 + HD,
                                                      128 * (t % 4):
                                                      128 * (t % 4) + 128]
                                else:
                                    ksrc = kt[(pg, c // 4)][r:r + HD,
                                                            128 * (c % 4):
                                                            128 * (c % 4) + 128]
                                    qsrc = qt[(pg, t // 4)][r:r + HD,
                                                            128 * (t % 4):
                                                            128 * (t % 4) + 128]
                                nc.tensor.matmul(
                                    out=sps[:, hi * 512 + j * 128:
                                            hi * 512 + (j + 1) * 128],
                                    lhsT=ksrc, rhs=qsrc,
                                    start=True, stop=True,
                                )
                        nc.scalar.activation(
                            out=pe_t[:, hp * 1024:(hp + 1) * 1024], in_=sps,
                            func=mybir.ActivationFunctionType.Exp,
                        )
                    # ONE masked multiply for all 4 head pairs: in2 repeats the
                    # class mask across pairs via a stride-0 middle AP dim
                    mk_ap = msk[:, cls * 1024:(cls + 1) * 1024]
                    nc.vector.tensor_mul(
                        pT.rearrange("p (g c) -> p g c", c=1024),
                        pe_t.rearrange("p (g c) -> p g c", c=1024),
                        bass.AP(tensor=mk_ap.tensor, offset=mk_ap.offset,
                                ap=[list(mk_ap.ap[0]), [0, 4], [1, 1024]]))
                    for hp2 in range(2):
                        # FOUR heads' AV into one PSUM bank: hh*(HD+1) col base
                        av = ps_av.tile([128, 4 * (HD + 1)], F32, name="av",
                                        tag="av")
                        for hh in range(4):
                            h = 4 * hp2 + hh
                            hp, hi = h // 2, h % 2
                            for j in range(4):
                                nc.tensor.matmul(
                                    out=av[:, hh * (HD + 1):hh * (HD + 1) + HD + 1],
                                    lhsT=pT[:, hp * 1024 + hi * 512 + j * 128:
                                            hp * 1024 + hi * 512 + (j + 1) * 128],
                                    rhs=vv[t + j][:, h * (HD + 1):
                                                  (h + 1) * (HD + 1)],
                                    start=(j == 0), stop=(j == 3),
                                )
                        # one recip over 4 rowsums, one broadcast-mul normalize
                        rec = wk.tile([128, 4], F32, name="rec", tag="rec")
                        nc.vector.reciprocal(
                            rec,
                            bass.AP(tensor=av.tensor, offset=av.offset + HD,
                                    ap=[list(av.ap[0]), [HD + 1, 4]]))
                        nc.vector.tensor_mul(
                            oall[:, hp2 * 128:(hp2 + 1) * 128]
                                .rearrange("p (g d) -> p g d", d=HD),
                            av.rearrange("p (g e) -> p g e", e=HD + 1)[:, :, 0:HD],
                            bass.AP(tensor=rec.tensor, offset=rec.offset,
                                    ap=[list(rec.ap[0]), [1, 4], [0, HD]]))
                    ypst = ps_y.tile([128, DIM + 128], F32, name="yps", tag="psy")
                    yps = ypst[:, 0:DIM]
                    tp = ypst[:, DIM:DIM + 128].bitcast(BF16)
                    for cg in range(2):
                        nc.tensor.transpose(
                            tp[:, cg * 128:(cg + 1) * 128],
                            oall[:, cg * 128:(cg + 1) * 128], idt)
                    oT = op.tile([128, 256], BF16, name="oT", tag="oT")
                    nc.vector.tensor_copy(out=oT, in_=tp)
                    nc.tensor.matmul(out=yps, lhsT=ones,
                                     rhs=bb[0:1, :],
                                     start=True, stop=False)
                    for cg in range(2):
                        nc.tensor.matmul(
                            out=yps,
                            lhsT=oT[:, cg * 128:(cg + 1) * 128],
                            rhs=wp[cg],
                            start=False, stop=(cg == 1),
                        )
                    yt = op.tile([128, DIM], F32, name="yt", tag="yt")
                    nc.vector.tensor_copy(out=yt, in_=yps)
                    nc.sync.dma_start(
                        out=y[b * N + t * 128:b * N + (t + 1) * 128, :], in_=yt)

                for g in range((NHB + 511) // 512):  # 9 groups
                    emit_k_group(g)
                    if g < N // 512:
                        emit_q_group(g)
                    for ch in range(4 * g, min(4 * g + 4, NCH)):
                        emit_v_chunk(ch)
                    if g >= 1:
                        for t in range(4 * (g - 1), 4 * g):
                            emit_tile(t)

    nc.compile()  # legalize waits (<=1 per instruction) for walrus
    return nc


_PROGRAM_CACHE: dict = {}


def _program() -> bass.Bass:
    if "nc" not in _PROGRAM_CACHE:
        _PROGRAM_CACHE["nc"] = _build_program()
    return _PROGRAM_CACHE["nc"]


def _masks() -> np.ndarray:
    """maskP[ki, cls*1024 + j*128 + qi] (duplicated at +512 for head pairs)."""
    import ml_dtypes
    m = np.zeros((128, 5 * 1024), ml_dtypes.bfloat16)
    ki = np.arange(128)
    qi = np.arange(128)
    kc = ki % GRID
    qc = qi % GRID
    for cls, t in enumerate(MREP):
        qr = 2 * t + qi // GRID
        for j in range(4):
            kr = 2 * (t + j) + ki // GRID - HALF
            valid = (
                (kr[:, None] >= 0) & (kr[:, None] < GRID)
                & (np.abs(kr[:, None] - qr[None, :]) <= HALF)
                & (np.abs(kc[:, None] - qc[None, :]) <= HALF)
            )
            m[:, cls * 1024 + j * 128:cls * 1024 + (j + 1) * 128] = valid
            m[:, cls * 1024 + 512 + j * 128:
              cls * 1024 + 512 + (j + 1) * 128] = valid
    return m


def _in_blob(x, W_qkv, W_proj, b_proj, temperature) -> np.ndarray:
    import ml_dtypes
    bf = ml_dtypes.bfloat16
    x = np.asarray(x, np.float32)
    wqkvT = np.ascontiguousarray(np.asarray(W_qkv, np.float32).T)
    wqkvT[:, :DIM] *= np.float32(SCALE) * np.float32(np.asarray(temperature)[0])

    blob = np.zeros((128, BLOB_COLS), bf)
    for cc in range(2):
        blob[:, OFF_WQKV + cc * 768:OFF_WQKV + (cc + 1) * 768] = \
            wqkvT[cc * 128:(cc + 1) * 128].astype(bf)
        blob[:, OFF_WP + cc * 256:OFF_WP + (cc + 1) * 256] = \
            np.asarray(W_proj, np.float32).T[cc * 128:(cc + 1) * 128].astype(bf)
    blob[0, OFF_BP:OFF_BP + 256] = np.asarray(b_proj, np.float32).astype(bf)
    blob[:, OFF_ID:OFF_ID + 128] = np.eye(128, dtype=np.float32)
    blob[:, OFF_MSK:OFF_MSK + 5 * 1024] = _masks()
    for b in range(B):
        xT = x[b].T.astype(bf)  # [256, 4096]
        for cc in range(2):
            c0 = OFF_X + cc * XCOLS + b * XSTEP + PAD
            blob[:, c0:c0 + N] = xT[cc * 128:(cc + 1) * 128]
    return blob


class _Runner:
    """Persistent single-core PJRT executable. Plain jit (no shard_map):
    measured ~130 us/call cheaper than a 1-device shard_map wrapper.
    All tensors are donated: blob is an aliased in-out (content readable on
    device, no per-call XLA copy), y is a plain donated output."""

    NCORES = 1

    def __init__(self, nc: bass.Bass):
        import jax
        from concourse import bass2jax
        from concourse import mybir as mb

        bass2jax.install_neuronx_cc_hook()
        self.jax = jax

        partition_name = (nc.partition_id_tensor.name
                          if nc.partition_id_tensor else None)
        in_names, out_names, out_avals, zero_outs = [], [], [], []
        for alloc in nc.m.functions[0].allocations:
            if not isinstance(alloc, mb.MemoryLocationSet):
                continue
            name = alloc.memorylocations[0].name
            if alloc.kind == "ExternalInput":
                if name != partition_name:
                    in_names.append(name)
            elif alloc.kind == "ExternalOutput":
                out_names.append(name)
                shape = tuple(alloc.tensor_shape)
                dtype = mb.dt.np(alloc.dtype)
                out_avals.append(jax.core.ShapedArray(shape, dtype))
                zero_outs.append(np.zeros(shape, dtype))
        self.in_names, self.out_names = in_names, out_names
        self.out_avals, self.zero_outs = out_avals, zero_outs
        n_params, n_outs = len(in_names), len(out_names)
        all_names = list(in_names + out_names)
        if partition_name is not None:
            all_names.append(partition_name)
        all_names = tuple(all_names)
        # every output aliases the operand that carries its donated buffer
        aliases = tuple((oi, n_params + oi) for oi in range(n_outs))

        def _body(*args):
            operands = list(args)
            if partition_name is not None:
                operands.append(bass2jax.partition_id_tensor())
            outs = bass2jax._bass_exec_p.bind(
                *operands,
                out_avals=tuple(out_avals),
                in_names=all_names,
                out_names=tuple(out_names),
                lowering_input_output_aliases=aliases,
                sim_require_finite=True,
                sim_require_nnan=True,
                nc=nc,
            )
            return tuple(outs)

        self.sharded = jax.jit(
            _body,
            donate_argnums=tuple(range(n_params, n_params + n_outs)),
            keep_unused=True,
        )

    def _operands(self, blob):
        """Donated operands in out_names order (blob data + zeroed outputs)."""
        ops = []
        for n, z in zip(self.out_names, self.zero_outs):
            ops.append(np.ascontiguousarray(blob) if n == "blob"
                       else np.zeros(z.shape, z.dtype))
        return ops

    def __call__(self, blob):
        out_arrs = self.sharded(*self._operands(blob))
        return {n: np.asarray(out_arrs[i]) for i, n in enumerate(self.out_names)}

    def bench(self, blob, iters: int = 200):
        """Steady-state per-iteration time (s) with pipelined dispatch."""
        import time
        jax = self.jax
        zs = [[jax.device_put(o) for o in self._operands(blob)]
              for _ in range(iters + 2)]
        for zz in zs:
            for z in zz:
                z.block_until_ready()
        # warmup
        outs = self.sharded(*zs[0])
        jax.block_until_ready(outs)
        outs = self.sharded(*zs[1])
        jax.block_until_ready(outs)
        t0 = time.monotonic()
        last = None
        for i in range(iters):
            last = self.sharded(*zs[2 + i])
        jax.block_until_ready(last)
        t1 = time.monotonic()
        return (t1 - t0) / iters


def _runner() -> _Runner:
    if "runner" not in _PROGRAM_CACHE:
        _PROGRAM_CACHE["runner"] = _Runner(_program())
    return _PROGRAM_CACHE["runner"]


def run(inputs: dict):
    """Returns (out [B,N,DIM] f32, raw result dict)."""
    blob = _in_blob(**inputs)
    result = _runner()(blob)
    out = np.ascontiguousarray(result["y"].reshape(B, N, DIM))
    return out, result


def kernel(x, W_qkv, W_proj, b_proj, temperature):
    out, _ = run({"x": x, "W_qkv": W_qkv, "W_proj": W_proj,
                  "b_proj": b_proj, "temperature": temperature})
    return out
